# revision 12
# baseline (speedup 1.0000x reference)
"""Causal grouped-query paged attention (prefill) on 8 Trainium2 NeuronCores.

Problem (hardcoded): T=4096 tokens (B=2 seqs x SEQ=2048), 32 q heads,
8 kv heads (GQA group g=4), head_dim=128, paged fp32 KV cache
(512 blocks x 16 tokens).

Sharding: tensor-parallel over KV heads -- core h gets kv head h, its 4
query heads, and both sequences => 8 causal attention slices of
[2048 q x 2048 k x 128 d] per core.

Device kernel design (per core), v2:
  - S^T orientation: scores^T[k, q] = K_chunk (lhsT) x Q^T (moving), so k
    lands on PSUM partitions and P@V needs no transposes.
  - score chunks are packed in PAIRS into [128, 1024] PSUM tiles (2 banks)
    so one ScalarE exp instruction covers two chunks (halves ScalarE
    instruction overhead, the co-bottleneck).
  - causal diagonal handled with tight widths (512/384/256/128) + a
    precomputed upper-tri 0/1 mask multiply on VectorE.
  - softmax denominator: P^T pairs are pre-summed on VectorE (fp16 2x
    mode), then a single ones-column matmul per pair streams into a
    [1, 512] PSUM row -- the denominator matmul stream drops ~60%.
  - normalization without transposes: VectorE reciprocal of the denom row
    -> 1-partition broadcast matmul ([1,128] ones row) -> one VectorE
    tensor_mul (PSUM x PSUM -> SBUF f32) -> DMA O^T out. The final
    O^T -> O transpose happens host-side (pure data movement).
  - 2-group QK lookahead in PE program order hides the exp latency;
    initial loads are split and spread over 4 DMA queues so the first
    matmul starts ~2us in instead of ~16us.

kernel(**inputs) does the paged cache scatter/gather + head sharding +
layout transposes host-side in numpy (pure data movement), runs the same
NEFF SPMD on cores 0-7, and reassembles the full [4096, 4096] output.
"""

import math

import numpy as np

import concourse.bass as bass
import concourse.tile as tile
from concourse import bacc, mybir
from concourse.bass_utils import run_bass_kernel_spmd

# problem constants
B = 2
SEQ = 2048
T = B * SEQ
N_QO_HEADS = 32
N_KV_HEADS = 8
G = N_QO_HEADS // N_KV_HEADS  # 4
D = 128
PAGE = 16
NUM_BLOCKS = 512
N_CORES = 8

QTILE = 512  # q chunk (matmul moving dim)
KCH = 128    # k chunk (contraction tile)
F32 = mybir.dt.float32
FP16 = mybir.dt.float16
SM_SCALE = 1.0 / math.sqrt(D)
EXP = mybir.ActivationFunctionType.Exp


def emit(nc, n_slices, n_seqs, seq, slice_to_seq):
    """Emit the attention program. Inputs (DRAM):
      qt   [n_slices, 128, seq]  Q^T per slice (fp16)
      kt   [n_seqs,   128, seq]  K^T per sequence (fp16)
      v    [n_seqs,   seq, 128]  V per sequence (fp16)
      mask [128, 128]            upper-tri ones (fp16)
      ones [128, 128]            all ones (fp16)
    Output: o [n_slices, 128, seq]  O^T (f32, unnormalized-then-normalized
            on device; host only transposes).
    """
    nq = seq // QTILE
    ndiag = QTILE // KCH  # 4

    qt = nc.dram_tensor("qt", [n_slices, D, seq], FP16, kind="ExternalInput").ap()
    kt = nc.dram_tensor("kt", [n_seqs, D, seq], FP16, kind="ExternalInput").ap()
    v = nc.dram_tensor("v", [n_seqs, seq, D], FP16, kind="ExternalInput").ap()
    mask = nc.dram_tensor("mask", [D, D], FP16, kind="ExternalInput").ap()
    ones = nc.dram_tensor("ones", [D, D], FP16, kind="ExternalInput").ap()
    o = nc.dram_tensor("o", [n_slices, D, seq], F32, kind="ExternalOutput").ap()

    with tile.TileContext(nc) as tc:
        with (
            tc.tile_pool(name="const", bufs=1) as const_pool,
            tc.tile_pool(name="kv", bufs=1) as kv_pool,
            tc.tile_pool(name="q", bufs=1) as q_pool,
            tc.tile_pool(name="pt", bufs=4) as pt_pool,
            tc.tile_pool(name="s2", bufs=2) as s2_pool,
            tc.tile_pool(name="rcp", bufs=2) as rcp_pool,
            tc.tile_pool(name="osb", bufs=2) as osb_pool,
            tc.tile_pool(name="bcs", bufs=2) as bcs_pool,
            tc.tile_pool(name="st", bufs=2, space="PSUM") as st_pool,
            tc.tile_pool(name="ot", bufs=2, space="PSUM") as ot_pool,
            tc.tile_pool(name="ds", bufs=1, space="PSUM") as ds_pool,
            tc.tile_pool(name="bc", bufs=1, space="PSUM") as bc_pool,
        ):
            mask_sb = const_pool.tile([D, D], FP16)
            ones_sb = const_pool.tile([D, D], FP16)
            kt_sb = []
            v_sb = []
            for b in range(n_seqs):
                kt_sb.append(kv_pool.tile([D, seq], FP16, tag=f"kt{b}", name=f"ktsb{b}"))
                v_sb.append(kv_pool.tile([D, seq], FP16, tag=f"v{b}", name=f"vsb{b}"))
            qt_sb = [
                q_pool.tile([D, seq], FP16, tag=f"qt{s}", name=f"qtsb{s}")
                for s in range(n_slices)
            ]

            # --- initial loads: split + spread across 4 DMA queues so the
            # first QK matmul can start after ~130KB instead of ~3.5MB.
            b0 = slice_to_seq[0]
            b1 = 1 - b0 if n_seqs == 2 else b0
            HEAD = 4 * KCH  # first four k-chunks (all of qc=0)
            # sync queue: K^T head for seq b0 (first 4 k-chunks cover all of
            # qc=0), then the rest of K^T
            nc.sync.dma_start(kt_sb[b0][:, 0:HEAD], kt[b0, :, 0:HEAD])
            # scalar queue: Q^T head for slice 0, V head for seq b0
            # (chunk-packed along free dim), then the rests
            nc.scalar.dma_start(qt_sb[0][:, 0:QTILE], qt[0, :, 0:QTILE])
            nc.sync.dma_start(kt_sb[b0][:, HEAD:seq], kt[b0, :, HEAD:seq])
            nc.scalar.dma_start(
                v_sb[b0][:, 0:HEAD].rearrange("p (c d) -> p c d", d=D),
                v[b0, 0:HEAD].rearrange("(c p) d -> p c d", p=D),
            )
            nc.scalar.dma_start(qt_sb[0][:, QTILE:seq], qt[0, :, QTILE:seq])
            nc.scalar.dma_start(
                v_sb[b0][:, HEAD:seq].rearrange("p (c d) -> p c d", d=D),
                v[b0, HEAD:seq].rearrange("(c p) d -> p c d", p=D),
            )
            # gpsimd queue (idle engine): constants, later slices' Q,
            # and the other sequence's K/V -- all needed much later.
            nc.gpsimd.dma_start(mask_sb[:], mask[:])
            nc.gpsimd.dma_start(ones_sb[:], ones[:])
            for s in range(1, 4):
                if s < n_slices:
                    nc.gpsimd.dma_start(qt_sb[s][:], qt[s])
            if n_seqs == 2:
                nc.gpsimd.dma_start(kt_sb[b1][:], kt[b1])
                nc.gpsimd.dma_start(
                    v_sb[b1][:].rearrange("p (c d) -> p c d", d=D),
                    v[b1].rearrange("(c p) d -> p c d", p=D),
                )  # noqa: the b1 seq is first needed at slice G (~half-way)
            for s in range(4, n_slices):
                nc.gpsimd.dma_start(qt_sb[s][:], qt[s])

            for s in range(n_slices):
                b = slice_to_seq[s]
                ktb = kt_sb[b]
                vb = v_sb[b]
                qts = qt_sb[s]
                for qc in range(nq):
                    q0 = qc * QTILE
                    nfull = ndiag * qc
                    base = nfull
                    # groups: full-chunk pairs, then two diagonal groups.
                    # each group: list of (kc, q_off, width, masked)
                    groups = []
                    for pi in range(nfull // 2):
                        groups.append(
                            [(2 * pi, 0, QTILE, False), (2 * pi + 1, 0, QTILE, False)]
                        )
                    groups.append(
                        [(base + 0, 0, 512, True), (base + 1, 128, 384, True)]
                    )
                    groups.append(
                        [(base + 2, 256, 256, True), (base + 3, 384, 128, True)]
                    )

                    ot_ps = ot_pool.tile([D, QTILE], F32)
                    ds_ps = ds_pool.tile([1, QTILE], F32)

                    ngroups = len(groups)
                    st_tiles = [None] * ngroups
                    pt_tiles = [None] * ngroups

                    def emit_qk(g):
                        st = st_pool.tile([D, 2 * QTILE], F32, name="st_ps")
                        col = 0
                        for kc, off, w, _m in groups[g]:
                            nc.tensor.matmul(
                                st[:, col : col + w],
                                lhsT=ktb[:, kc * KCH : (kc + 1) * KCH],
                                rhs=qts[:, q0 + off : q0 + off + w],
                                start=True,
                                stop=True,
                            )
                            col += w
                        st_tiles[g] = (st, col)

                    # 2-group lookahead so PE never waits on exp
                    emit_qk(0)
                    if ngroups > 1:
                        emit_qk(1)

                    pv_emitted = 0
                    npv = nfull + 4
                    ds_emitted = 0
                    nds = nfull // 2 + 4

                    for g in range(ngroups):
                        if g + 2 < ngroups:
                            emit_qk(g + 2)
                        st, totw = st_tiles[g]
                        ptile = pt_pool.tile([D, 2 * QTILE], FP16, name="ptile")
                        pt_tiles[g] = ptile
                        nc.scalar.activation(
                            ptile[:, 0:totw], st[:, 0:totw], EXP, scale=SM_SCALE
                        )
                        # causal masks (diagonal 128-col block of each chunk)
                        col = 0
                        for kc, off, w, m in groups[g]:
                            if m:
                                nc.vector.tensor_mul(
                                    ptile[:, col : col + KCH],
                                    ptile[:, col : col + KCH],
                                    mask_sb[:],
                                )
                            col += w
                        # P @ V accumulation
                        col = 0
                        for kc, off, w, _m in groups[g]:
                            nc.tensor.matmul(
                                ot_ps[:, off : off + w],
                                lhsT=vb[:, kc * KCH : (kc + 1) * KCH],
                                rhs=ptile[:, col : col + w],
                                start=(pv_emitted == 0),
                                stop=(pv_emitted == npv - 1),
                            )
                            pv_emitted += 1
                            col += w
                        # denominator contributions
                        (kc_a, off_a, w_a, m_a) = groups[g][0]
                        (kc_b, off_b, w_b, m_b) = groups[g][1]
                        # `start` must be True for the FIRST matmul touching
                        # each ds region: with no full pairs (qc==0), the
                        # first two diagonal pieces cover [0:128] and
                        # [128:512] and both need start=True.
                        if not m_a:
                            # full pair: presum on DVE, one ds matmul
                            s2 = s2_pool.tile([D, QTILE], FP16, name="s2t")
                            nc.vector.tensor_add(
                                s2[:, 0:QTILE],
                                ptile[:, 0:QTILE],
                                ptile[:, QTILE : 2 * QTILE],
                            )
                            nc.tensor.matmul(
                                ds_ps[0:1, 0:QTILE],
                                lhsT=ones_sb[:, 0:1],
                                rhs=s2[:, 0:QTILE],
                                start=(ds_emitted == 0),
                                stop=(ds_emitted == nds - 1),
                            )
                            ds_emitted += 1
                        else:
                            # diagonal pair (widths w_a > w_b, offsets
                            # off_b = off_a + 128): chunk A's first 128
                            # cols stand alone; the overlap [off_b, off_a+w_a)
                            # is presummed with chunk B.
                            ov = w_a - KCH  # overlap width == w_b
                            nc.tensor.matmul(
                                ds_ps[0:1, off_a : off_a + KCH],
                                lhsT=ones_sb[:, 0:1],
                                rhs=ptile[:, 0:KCH],
                                start=(ds_emitted == 0),
                                stop=(ds_emitted == nds - 1),
                            )
                            ds_emitted += 1
                            s2 = s2_pool.tile([D, QTILE], FP16, name="s2t")
                            nc.vector.tensor_add(
                                s2[:, 0:ov],
                                ptile[:, KCH : KCH + ov],
                                ptile[:, w_a : w_a + w_b],
                            )
                            nc.tensor.matmul(
                                ds_ps[0:1, off_b : off_b + ov],
                                lhsT=ones_sb[:, 0:1],
                                rhs=s2[:, 0:ov],
                                start=(ds_emitted == 0)
                                or (nfull == 0 and ds_emitted == 1),
                                stop=(ds_emitted == nds - 1),
                            )
                            ds_emitted += 1

                    # epilogue: normalize O^T by 1/denom and DMA out.
                    rcp = rcp_pool.tile([1, QTILE], FP16)
                    with nc.allow_low_precision(
                        reason="1/denom in fp16: 5e-4 rel, well within 2e-2"
                    ):
                        nc.vector.reciprocal(rcp[0:1, :], ds_ps[0:1, :])
                    bc_ps = bc_pool.tile([D, QTILE], F32)
                    nc.tensor.matmul(
                        bc_ps[:],
                        lhsT=ones_sb[0:1, :],
                        rhs=rcp[0:1, :],
                        start=True,
                        stop=True,
                    )
                    # DVE can read only one PSUM operand per instruction:
                    # bounce the broadcast rows through SBUF first.
                    bc_sb = bcs_pool.tile([D, QTILE], F32)
                    nc.vector.tensor_copy(bc_sb[:], bc_ps[:])
                    o_sb = osb_pool.tile([D, QTILE], F32)
                    nc.vector.tensor_mul(o_sb[:], ot_ps[:], bc_sb[:])
                    nc.sync.dma_start(o[s, :, q0 : q0 + QTILE], o_sb[:])
    return nc


_CACHE = {}


def _build_full():
    key = "full"
    if key not in _CACHE:
        nc = bacc.Bacc(
            "TRN2",
            target_bir_lowering=False,
            debug=False,
            enable_asserts=False,
            num_devices=N_CORES,
        )
        emit(nc, n_slices=B * G, n_seqs=B, seq=SEQ,
             slice_to_seq=[b for b in range(B) for _ in range(G)])
        nc.compile()
        _CACHE[key] = nc
    return _CACHE[key]


def make_mask():
    return np.triu(np.ones((D, D), dtype=np.float16))


def shard_inputs(query, key, value, key_cache, value_cache, block_tables,
                 new_cache_slots):
    """Host-side scatter/gather + head sharding. Returns per-core input maps."""
    kc = key_cache.reshape(NUM_BLOCKS * PAGE, N_KV_HEADS, D).copy()
    vc = value_cache.reshape(NUM_BLOCKS * PAGE, N_KV_HEADS, D).copy()
    kc[new_cache_slots] = key.reshape(T, N_KV_HEADS, D)
    vc[new_cache_slots] = value.reshape(T, N_KV_HEADS, D)
    nb = block_tables.shape[1]
    idx = (
        block_tables[:, :, None].astype(np.int64) * PAGE
        + np.arange(PAGE, dtype=np.int64)[None, None, :]
    ).reshape(B, SEQ)
    k_all = kc[idx]  # [B, SEQ, Hkv, D]
    v_all = vc[idx]
    q_all = query.reshape(B, SEQ, N_KV_HEADS, G, D)
    mask = make_mask()

    bf = np.float16
    in_maps = []
    for h in range(N_CORES):
        qt = np.ascontiguousarray(
            q_all[:, :, h, :, :].transpose(0, 2, 3, 1).reshape(B * G, D, SEQ)
        ).astype(bf)
        kt = np.ascontiguousarray(k_all[:, :, h, :].transpose(0, 2, 1)).astype(bf)
        vv = np.ascontiguousarray(v_all[:, :, h, :]).astype(bf)
        in_maps.append({"qt": qt, "kt": kt, "v": vv, "mask": mask,
                        "ones": np.ones((D, D), dtype=bf)})
    return in_maps


def assemble_output(results):
    out = np.empty((B, SEQ, N_KV_HEADS, G, D), dtype=np.float32)
    for h in range(N_CORES):
        oc = results[h]["o"].reshape(B, G, D, SEQ)
        out[:, :, h, :, :] = oc.transpose(0, 3, 1, 2)
    return out.reshape(T, N_QO_HEADS * D)


def kernel(query, key, value, key_cache, value_cache, block_tables,
           new_cache_slots, _trace=False):
    query = np.asarray(query, dtype=np.float32)
    key = np.asarray(key, dtype=np.float32)
    value = np.asarray(value, dtype=np.float32)
    key_cache = np.asarray(key_cache, dtype=np.float32)
    value_cache = np.asarray(value_cache, dtype=np.float32)
    block_tables = np.asarray(block_tables)
    new_cache_slots = np.asarray(new_cache_slots)

    nc = _build_full()
    in_maps = shard_inputs(query, key, value, key_cache, value_cache,
                           block_tables, new_cache_slots)
    res = run_bass_kernel_spmd(
        nc, in_maps, core_ids=list(range(N_CORES)), trace=_trace
    )
    out = assemble_output(res.results)
    if _trace:
        kernel.last_result = res
    return out
